# revision 1
# baseline (speedup 1.0000x reference)
"""Trainium2 SPMD kernel for batched B-spline basis evaluation.

Math: for cubic B-splines on a uniform 12-point grid, the spline space on the
x-domain [grid[3], grid[8]) is spanned exactly by
    {1, x, x^2, x^3, relu(x-k)^3 for k in {grid[5], grid[6]},
     relu(k-x)^3 for k in {grid[4], grid[5]}... }
i.e. a cubic polynomial plus one outward-pointing truncated cubic per interior
knot (grid[4..7]).  "Outward" (forward-truncated for positive knots,
backward-truncated for negative ones) keeps every term zero near the center,
which bounds the term magnitudes and the cancellation error.

So  out[b, s] = w0[s] + w1[s]*x + w2[s]*x^2 + w3[s]*x^3
             + sum_l v_l[s] * relu(sg_l*(x - k_l))^3        (sg_l = +-1)

The per-spline weights (w*, v*) are a tiny (S x 8) @ (8 x 8) host-side
transform of `coefficients`.  On device, per 128-spline partition tile:
    z_l  = relu(sg_l*x - sg_l*k_l)           (ScalarE Relu, per-partition bias)
    q_l  = z_l^2                             (ScalarE Square)
    vz_l = v_l[s] * z_l                      (VectorE tensor_scalar, per-partition)
    m_l  = q_l * vz_l                        (VectorE tensor_tensor)
    out  = poly + sum_l m_l                  (VectorE adds, bf16 result)
with x transposed on-chip via TensorE identity-matmuls so splines sit on
partitions and per-spline weights ride per-partition scalar operands; the
bf16 result is transposed back on TensorE and cast to f32 by the SWDGE
store DMA.  (SBUF->SBUF xbar DMA-transposes deadlock the chip when all 8
cores issue them concurrently, and GpSimd compute offload measurably slows
the schedule, so both are avoided.)

Toolchain notes baked in below: every compute/DMA ISA struct on this walrus
holds a single sync-wait (EventSemaphore carriers hold 2), so
_split_overflow_waits() legalizes the scheduled IR, plus one-wait absorbers
for the PE dependency fan-in.

Sharding: data-parallel over the spline axis (columns) across 8 cores; each
core handles 512 splines x full batch.  No collectives needed.
"""

import os
import numpy as np
import ml_dtypes
from contextlib import ExitStack

# ---------------------------------------------------------------- dimensions
B = 4096          # batch rows
S = 4096          # splines
NCORES = 8
SPLINE_ORDER = 3

# Device tiling (overridable for small-size simulation)
NH = 4            # batch is processed in NH row-groups

LAST_EXEC_TIME_NS = None
_GRAPH_CACHE = {}


# ------------------------------------------------------------- host math
def _bspline_bases_np(x, grid, eps=1e-8):
    """Float64 replica of the reference Cox-de Boor recursion."""
    x = x.astype(np.float64)[:, None]
    g = grid.astype(np.float64)
    bases = ((x >= g[:-1]) & (x < g[1:])).astype(np.float64)
    for k in range(1, SPLINE_ORDER + 1):
        left = (x - g[: -(k + 1)]) / (g[k:-1] - g[: -(k + 1)] + eps)
        right = (g[k + 1:] - x) / (g[k + 1:] - g[1:-k] + eps)
        bases = left * bases[:, :-1] + right * bases[:, 1:]
    return bases


def _knot_spec(grid):
    """Outward-pointing truncated-power knots: [(k, sign), ...] for grid[4..7]."""
    g = grid.astype(np.float64)
    lo, hi = g[SPLINE_ORDER], g[len(g) - 1 - SPLINE_ORDER]
    mid = 0.5 * (lo + hi)
    spec = []
    for k in g[SPLINE_ORDER + 1: len(g) - 1 - SPLINE_ORDER]:
        sg = 1.0 if k >= mid else -1.0     # forward for right-half knots
        spec.append((float(k), sg))
    return spec, float(lo), float(hi)


def _coeffs_to_weights(coefficients, grid):
    """(S, n_basis) coefficients -> (S, 4+n_knots) trunc-power weights."""
    spec, lo, hi = _knot_spec(grid)
    xs = np.linspace(lo, hi - 1e-6, 4001)
    feats = [np.ones_like(xs), xs, xs * xs, xs ** 3]
    for k, sg in spec:
        feats.append(np.maximum(sg * (xs - k), 0.0) ** 3)
    Phi = np.stack(feats, axis=1)
    Bas = _bspline_bases_np(xs, grid)
    M, *_ = np.linalg.lstsq(Phi, Bas, rcond=None)   # Bas ~= Phi @ M
    W = coefficients.astype(np.float64) @ M.T
    # append per-knot bias columns (-sg*k) for the ACT Relu z-computation
    kb = np.array([-sg * k for k, sg in spec], dtype=np.float64)
    W = np.concatenate([W, np.broadcast_to(kb, (W.shape[0], len(spec)))], axis=1)
    return np.ascontiguousarray(W), spec


# ------------------------------------------------------------- device graph
def _build_graph(knot_spec, b, ss):
    from concourse import bass, tile, mybir, masks
    

    f32 = mybir.dt.float32
    bf16 = mybir.dt.bfloat16
    nch = ss // 128           # spline chunks per core
    bh = b // NH              # rows per group
    nblk = bh // 128          # 128-row blocks per group
    nk = len(knot_spec)
    nw = 4 + 2 * nk           # [w0..w3, v0..v_{nk-1}, kb0..kb_{nk-1}]

    nc = bass.Bass()
    x_d = nc.declare_dram_parameter("x", [b, ss], f32, isOutput=False)
    w_d = nc.declare_dram_parameter("w", [128, nch, nw], f32, isOutput=False)
    out_d = nc.declare_dram_parameter("out", [b, ss], f32, isOutput=True)

    with tile.TileContext(nc) as tc, ExitStack() as ctx:
        cpool = ctx.enter_context(tc.tile_pool(name="consts", bufs=1))
        W = cpool.tile([128, nch, nw], f32)
        nc.sync.dma_start(out=W[:], in_=w_d[:])
        ID = cpool.tile([128, 128], f32)
        masks.make_identity(nc, ID[:])
        IDb = cpool.tile([128, 128], bf16)
        masks.make_identity(nc, IDb[:])

        natp = ctx.enter_context(tc.tile_pool(name="nat", bufs=2))
        psp = ctx.enter_context(tc.tile_pool(name="ps", bufs=4, space="PSUM"))
        psbp = ctx.enter_context(tc.tile_pool(name="psb", bufs=4, space="PSUM"))
        # The PE ldweights ISA struct supports only one sync-wait, so every
        # external dependency PE picks up is absorbed by a throwaway
        # standalone ldweights that carries exactly that one wait.  (The
        # loaded garbage is overwritten by each real matmul's own load.)
        def pe_absorb(src):
            nc.tensor.ldweights(src.bitcast(bf16)[:, 0:128])

        pe_absorb(ID[:])   # absorb the make_identity (gpsimd) dependency

        # DVE-side absorber: the TS-with-scalar-pointer struct also allows a
        # single sync-wait, so the W DMA dependency must reach DVE alone.
        wscr = cpool.tile([128, 1], f32)
        nc.vector.tensor_copy(wscr[:], W[:, 0, 0:1])
        xtp = ctx.enter_context(tc.tile_pool(name="xt", bufs=3))
        imp = ctx.enter_context(tc.tile_pool(name="im", bufs=2))
        otp = ctx.enter_context(tc.tile_pool(name="ot", bufs=3))
        onp = ctx.enter_context(tc.tile_pool(name="on", bufs=2))

        for h in range(NH):
            nat = natp.tile([128, nblk, ss], f32, tag="nat")
            rows = x_d[h * bh:(h + 1) * bh, :].rearrange("(k p) s -> p k s", p=128)
            for c4 in range(nch):
                cs4 = slice(c4 * 128, (c4 + 1) * 128)
                nc.sync.dma_start(out=nat[:, :, cs4], in_=rows[:, :, cs4])
                pe_absorb(nat[:, 0, cs4])   # absorb this slab's DMA dep

            onat = onp.tile([128, nblk, ss], bf16, tag="onat")

            for c in range(nch):
                cs = slice(c * 128, (c + 1) * 128)
                # --- transpose x block into [spline, batch] layout (f32, PE)
                XT = xtp.tile([128, bh], f32, tag="xt")
                for q0 in range(0, nblk, 4):
                    qn = min(4, nblk - q0)
                    ps = psp.tile([128, 512], f32, tag="ps", name="ps")
                    for j in range(qn):
                        nc.tensor.transpose(
                            out=ps[:, j * 128:(j + 1) * 128],
                            in_=nat[:, q0 + j, cs],
                            identity=ID[:],
                        )
                    nc.scalar.copy(out=XT[:, q0 * 128:(q0 + qn) * 128],
                                   in_=ps[:, :qn * 128])

                def wcol(t):
                    return W[:, c, t:t + 1]

                # --- cubic polynomial part: t01 + x^2*(w2 + w3*x)
                t01 = imp.tile([128, bh], f32, tag="t01")
                nc.scalar.activation(out=t01[:], in_=XT[:],
                                     func=mybir.ActivationFunctionType.Identity,
                                     bias=wcol(0), scale=wcol(1))
                G = imp.tile([128, bh], f32, tag="G")
                nc.scalar.activation(out=G[:], in_=XT[:],
                                     func=mybir.ActivationFunctionType.Identity,
                                     bias=wcol(2), scale=wcol(3))
                X2 = imp.tile([128, bh], f32, tag="X2")
                nc.scalar.activation(out=X2[:], in_=XT[:],
                                     func=mybir.ActivationFunctionType.Square)
                P2 = imp.tile([128, bh], f32, tag="P2")
                nc.vector.tensor_tensor(out=P2[:], in0=X2[:], in1=G[:],
                                        op=mybir.AluOpType.mult)
                accp = imp.tile([128, bh], f32, tag="accp")
                nc.vector.tensor_tensor(out=accp[:], in0=P2[:], in1=t01[:],
                                        op=mybir.AluOpType.add)

                # --- truncated-power knot terms (v1 op forms):
                #     z = relu(sg*(x-k)); m = (z^2) * (v*z)
                ms = []
                for l, (kv, sg) in enumerate(knot_spec):
                    z = imp.tile([128, bh], f32, tag=f"z{l % 2}", name=f"z{l}")
                    nc.scalar.activation(
                        out=z[:], in_=XT[:],
                        func=mybir.ActivationFunctionType.Relu,
                        bias=wcol(4 + nk + l), scale=float(sg))
                    q = imp.tile([128, bh], f32, tag=f"q{l % 2}", name=f"q{l}")
                    nc.scalar.activation(out=q[:], in_=z[:],
                                         func=mybir.ActivationFunctionType.Square)
                    vz = imp.tile([128, bh], f32, tag=f"vz{l % 2}", name=f"vz{l}")
                    nc.vector.tensor_scalar(out=vz[:], in0=z[:], scalar1=wcol(4 + l),
                                            scalar2=None, op0=mybir.AluOpType.mult)
                    m = imp.tile([128, bh], f32, tag=f"m{l % 2}", name=f"m{l}")
                    nc.vector.tensor_tensor(out=m[:], in0=q[:], in1=vz[:],
                                            op=mybir.AluOpType.mult)
                    ms.append(m)

                # --- sum the 5 terms (pair adds on gpsimd, final on DVE)
                s1 = imp.tile([128, bh], f32, tag="s1")
                nc.vector.tensor_tensor(out=s1[:], in0=ms[0][:], in1=ms[1][:],
                                        op=mybir.AluOpType.add)
                s2 = imp.tile([128, bh], f32, tag="s2")
                nc.vector.tensor_tensor(out=s2[:], in0=ms[2][:], in1=ms[3][:],
                                        op=mybir.AluOpType.add)
                acc2 = imp.tile([128, bh], f32, tag="acc2")
                nc.vector.tensor_tensor(out=acc2[:], in0=accp[:], in1=s1[:],
                                        op=mybir.AluOpType.add)
                OT = otp.tile([128, bh], bf16, tag="ot")
                nc.vector.tensor_tensor(out=OT[:], in0=acc2[:], in1=s2[:],
                                        op=mybir.AluOpType.add)

                # --- transpose back (bf16, PE) into the natural-layout tile.
                # (SBUF->SBUF xbar DMA-transposes deadlock the chip when all
                # 8 cores run them concurrently, so PE does this side too.)
                for q0 in range(0, nblk, 4):
                    qn = min(4, nblk - q0)
                    psb = psbp.tile([128, 4, 128], bf16, tag="psb", name="psb")
                    for j in range(qn):
                        nc.tensor.transpose(
                            out=psb[:, j],
                            in_=OT[:, (q0 + j) * 128:(q0 + j + 1) * 128],
                            identity=IDb[:],
                        )
                    nc.scalar.copy(out=onat[:, q0:q0 + qn, cs],
                                   in_=psb[:, :qn])

                # SWDGE cast-DMA (bf16 -> f32) for this chunk's columns
                rows_o = out_d[h * bh:(h + 1) * bh, :].rearrange(
                    "(k p) s -> p k s", p=128)
                nc.gpsimd.dma_start(out=rows_o[:, :, cs], in_=onat[:, :, cs])


    return nc


# Engine-sequencer instruction types whose ISA structs hold only ONE
# sync-wait slot on this toolchain's walrus.
_ONE_WAIT_TYPES = (
    "InstTensorScalarPtr", "InstTensorTensor", "InstTensorScalar",
    "InstActivation", "InstMatmult", "InstLdweights", "InstCustomDveAnt",
    "InstTensorCopy", "InstCopy", "InstTensorReduce", "InstMemset",
    "InstStreamTranspose", "InstCopyPredicated",
    "InstTensorScalarAffineSelect", "InstReciprocal", "InstIota",
    "InstTensorTensorScan", "InstSemaphoreOp", "InstNop",
    "InstDmaTransposeAnt", "InstDMACopy", "InstDrain",
)


def _split_overflow_waits(nc):
    """Move overflow sync-waits onto InstEventSemaphore carriers (which hold
    up to 2 waits) inserted just before the overloaded instruction, same
    engine.  Works around the scheduler emitting more waits than the
    compute-instruction ISA structs can encode."""
    from concourse import mybir

    n_split = 0
    for f in nc.m.functions:
        for blk in f.blocks:
            newlist = []
            for ins in blk.instructions:
                si = ins.sync_info
                waits = list(si.on_wait) if (si is not None and si.on_wait) else []
                if type(ins).__name__ in _ONE_WAIT_TYPES and len(waits) > 1:
                    overflow, keep = waits[:-1], waits[-1:]
                    for i in range(0, len(overflow), 2):
                        chunk = overflow[i:i + 2]
                        ev = mybir.InstEventSemaphore(
                            name=f"{ins.name}-waitcarrier-{i}",
                            engine=ins.engine,
                            ins=[],
                            outs=[],
                            sync_info=mybir.SyncInfo(on_wait=chunk, on_update=[]),
                        )
                        newlist.append(ev)
                    ins.sync_info = mybir.SyncInfo(
                        on_wait=keep, on_update=list(si.on_update or []))
                    n_split += 1
                newlist.append(ins)
            blk.instructions = newlist
    return n_split


def _get_graph(knot_spec, b, ss):
    key = (tuple(knot_spec), b, ss, NH)
    if key not in _GRAPH_CACHE:
        nc = _build_graph(knot_spec, b, ss)
        _split_overflow_waits(nc)   # HW-path legalization (sim can't run these)
        _GRAPH_CACHE[key] = nc
    return _GRAPH_CACHE[key]


# ------------------------------------------------------------- profiling
def _ensure_ntff_hook():
    """Inject antenv.axon_hooks (absent in this image) so that
    run_bass_kernel_spmd(trace=True) can capture NTFF profiles via the
    axon PJRT .so."""
    import sys
    import types
    try:
        from antenv.axon_hooks import get_axon_ntff_profile_hook  # noqa: F401
        return True
    except ImportError:
        pass
    try:
        from trn_agent_boot.trn_boot import _ntff_profile_via_ctypes
        so_path = "/opt/axon/libaxon_pjrt.so"
        hook = _ntff_profile_via_ctypes(so_path)
        if hook is None:
            return False
        mod = types.ModuleType("antenv.axon_hooks")
        mod._hook = hook
        mod.get_axon_ntff_profile_hook = lambda: mod._hook
        mod.set_axon_ntff_profile_hook = lambda h: setattr(mod, "_hook", h)
        sys.modules["antenv.axon_hooks"] = mod
        import antenv
        antenv.axon_hooks = mod
        return True
    except Exception as e:  # degrade to trace-less run
        print(f"ntff hook injection failed: {e}")
        return False


# ------------------------------------------------------------- entry point
def kernel(x, coefficients, grid):
    global LAST_EXEC_TIME_NS
    from concourse.bass_utils import run_bass_kernel_spmd

    x = np.asarray(x, dtype=np.float32)
    coefficients = np.asarray(coefficients, dtype=np.float32)
    grid = np.asarray(grid, dtype=np.float32)
    b, s = x.shape
    ss = s // NCORES

    Wfull, spec = _coeffs_to_weights(coefficients, grid)   # (S, nw) float64
    nw = Wfull.shape[1]

    nc = _get_graph(spec, b, ss)

    in_maps = []
    for d in range(NCORES):
        sl = slice(d * ss, (d + 1) * ss)
        Wd = Wfull[sl].astype(np.float32)                  # (ss, nw)
        # device layout [128, nch, nw] with spline = chunk*128 + partition
        Wd = np.ascontiguousarray(
            Wd.reshape(ss // 128, 128, nw).transpose(1, 0, 2))
        in_maps.append({
            "x": np.ascontiguousarray(x[:, sl]),
            "w": Wd,
        })

    trace = bool(int(os.environ.get("BASS_SPLINE_TRACE", "0")))
    if trace:
        trace = _ensure_ntff_hook()
    res = run_bass_kernel_spmd(nc, in_maps, core_ids=list(range(NCORES)),
                               trace=trace)
    LAST_EXEC_TIME_NS = res.exec_time_ns
    out = np.concatenate([r["out"] for r in res.results], axis=1)
    return out.astype(np.float32)

